# revision 24
# baseline (speedup 1.0000x reference)
"""Trainium2 Bass kernel for nn_Attention_45037027066352 (sparse_attention).

Reference computation (per batch b, head h; N=1024 tokens, HD=64, H=12):
    qkv   = x @ Wqkv.T                     -> q,k,v [B,H,N,HD]
    Qspk  = relu(q) @ Wfc1.T + bfc1
    Kspk  = relu(k) @ Wfc2.T + bfc2
    att   = softmax(relu(Qspk @ Kspk.T * SCALE) * 2)
    out_h = att @ (relu(v) * 4)
    y     = concat_h(out_h) @ Wproj.T + bproj

Sharding: pure data-parallel over B=8 across the 8 NeuronCores (one batch
element per core); all weights replicated, no collectives.

Schedule: the exp of the 12 NxN score matrices (~12.6M elements) is a hard
~110us floor on the Scalar/ACT engine; PE matmul work is ~125us warm.  The
kernel software-pipelines head pairs so ACT never starves and the PE never
idles >2us (keeps the HAM clock gate at 8/8 = 2.4 GHz):

    qk(p0) fc0 sc0 | qk(p1) fc1 sc1 |
    [ vchunk(p) rs(p) pv(p) qk(p+2) fc(p+2) sc(p+2) ] for p = 0..5 | proj

  - q,k projections emit per head pair (m-major weight DMA) so pair-0
    scores reach ACT ~12us after launch.
  - v projection is computed in per-pair column chunks (vchunk) placed just
    before the pair's PV, keeping the early PE window free for scores.
  - rowsum (ones-matmul) + PV trail each pair's exp/max stream.
  - output projection runs in bf16 (was fp32r = 1/4 PE rate).

Per-core layout (as in the original kernel):
  - host pre-transposes x[b] -> xT [C,N] and Wqkv -> WqkvT [C,3C] so the
    contraction dim lands on SBUF partitions with no on-chip transposes.
  - q,k produced transposed qT/kT [768,N] (head pairs on partitions), v in
    natural layout [N,768].
  - scores come out as S^T [j,i]; P = exp(relu(z)) = max(exp(z),1): exp on
    ACT straight from PSUM, max(.,1) on DVE into bf16 (4x mode). Row sums
    ride as ones-matmuls packed A/B; reciprocal runs on a small [128,16]
    reshape (DRAM bounce) and is applied during the PV PSUM->SBUF copyback.
  - PE array packing: head pairs run concurrently on disjoint 64-row /
    64-col tile positions (auto-derived from base partitions).

TRN2 Matmult instructions encode at most ONE sync wait, so every matmul's
dependencies must either be pre-observed by the PE or share one semaphore:
  - each input DMA is "gated" by a tiny PE matmul reading it (PE then has
    observed that DMA queue's semaphore), and
  - every PSUM tile gets a 1-element DVE memset as its first toucher, with
    all PSUM->SBUF copybacks also on DVE, so matmuls wait only on DVE.
"""

import numpy as np

import concourse.bass as bass
import concourse.bacc as bacc_mod
import concourse.bass_isa as bass_isa
import concourse.mybir as mybir
import concourse.tile as tile
from concourse.bass_utils import run_bass_kernel_spmd

import ml_dtypes

B, N, C, H, HD = 8, 1024, 768, 12, 64
SCALE = HD**-0.5
T_STEPS = 4
N_HALF = T_STEPS // 2  # att accumulated N_HALF times; V accumulated T times

F32 = mybir.dt.float32
BF16 = mybir.dt.bfloat16

NPAIR = H // 2  # 6 head pairs
KC = C // 128  # 6 contraction chunks for C=768
NT = N // 128  # 8 token tiles
NH = N // 512  # 2 free-dim halves


def build_nc() -> bass.Bass:
    nc = bacc_mod.Bacc()

    xT = nc.dram_tensor("xT", [C, N], BF16, kind="ExternalInput")
    wqkvT = nc.dram_tensor("wqkvT", [C, 3 * C], BF16, kind="ExternalInput")
    wfc1p = nc.dram_tensor("wfc1p", [128, 128], BF16, kind="ExternalInput")
    wfc2p = nc.dram_tensor("wfc2p", [128, 128], BF16, kind="ExternalInput")
    b1p = nc.dram_tensor("b1p", [128, 1], F32, kind="ExternalInput")
    b2p = nc.dram_tensor("b2p", [128, 1], F32, kind="ExternalInput")
    wprojT = nc.dram_tensor("wprojT", [C, C], BF16, kind="ExternalInput")
    bprojp = nc.dram_tensor("bprojp", [128, KC], F32, kind="ExternalInput")

    yT = nc.dram_tensor("yT", [C, N], F32, kind="ExternalOutput")

    # scratch for the rowsum -> reciprocal reshape round trip
    rs_dram = nc.dram_tensor("rs_scratch", [NPAIR, 2, N], F32)
    rec_dram = nc.dram_tensor("rec_scratch", [NPAIR, 2, N], BF16)

    xT_v = xT.rearrange("(ko p) n -> p ko n", p=128)
    # m-major view of the q,k part of WqkvT: m indexes 128-col blocks
    wqkvT_m = wqkvT.rearrange("(ko p) (m j) -> p ko m j", p=128, j=128)
    wqkvT_v = wqkvT.rearrange("(ko p) j -> p ko j", p=128)
    wprojT_v = wprojT.rearrange("(ko p) e -> p ko e", p=128)
    yT_v = yT.rearrange("(eo p) n -> p eo n", p=128)

    with tile.TileContext(nc) as tc:
        with (
            tc.tile_pool(name="consts", bufs=1) as consts,
            tc.tile_pool(name="psum", bufs=3, space="PSUM") as psum,
            tc.tile_pool(name="pvps", bufs=2, space="PSUM") as pv_psum,
            tc.tile_pool(name="xin", bufs=1) as x_pool,
            tc.tile_pool(name="wqk", bufs=1) as wqk_pool,
            tc.tile_pool(name="wv", bufs=1) as wv_pool,
            tc.tile_pool(name="wproj", bufs=1) as wproj_pool,
            tc.tile_pool(name="vr", bufs=1) as vr_pool,
            tc.tile_pool(name="rqk", bufs=1) as rqk_pool,
            tc.tile_pool(name="spk", bufs=4) as spk_pool,
            tc.tile_pool(name="texp", bufs=3) as t_pool,
            tc.tile_pool(name="pt", bufs=4) as pt_pool,
            tc.tile_pool(name="outT", bufs=1) as outT_pool,
            tc.tile_pool(name="rsmisc", bufs=2) as rs_pool,
            tc.tile_pool(name="yt", bufs=2) as y_pool,
        ):
            trash_holder = [pv_psum.tile([128, 512], F32, tag="pv", name="trash")]

            def ps_tile():
                # PSUM tile whose slot-handoff waits land on a cheap DVE
                # memset (Matmult instructions only encode one sync wait).
                t = psum.tile([128, N], F32, tag="ps")
                nc.vector.memset(t[:, 0:1], 0.0)
                return t

            def pv_tile():
                t = pv_psum.tile([128, 512], F32, tag="pv")
                nc.vector.memset(t[:, 0:1], 0.0)
                return t

            def gate(region, kpart=128):
                # Tiny PE matmul reading a freshly DMA'd SBUF region so the
                # PE observes that DMA queue's semaphore once, instead of
                # each consuming matmul carrying its own DMA wait.
                m = 65 if kpart == 128 else 64
                nc.tensor.matmul(
                    trash_holder[0][0:m, 0:2],
                    lhsT=region[0:kpart, 0:m],
                    rhs=region[0:kpart, 0:2],
                    start=True,
                    stop=True,
                )

            # ---- SBUF buffers ----
            wfc1_sb = consts.tile([128, 128], BF16)  # blockdiag(Wfc1.T*2s, ..)
            wfc2_sb = consts.tile([128, 128], BF16)
            b1_sb = consts.tile([128, 1], F32)
            b2_sb = consts.tile([128, 1], F32)
            bproj_sb = consts.tile([128, KC], F32)
            ones_sb = consts.tile([128, HD], BF16)

            x_sb = x_pool.tile([128, KC, N], BF16)
            wqk_sb = wqk_pool.tile([128, 2 * NPAIR, KC, 128], BF16)
            wv_sb = wv_pool.tile([128, KC, C], BF16)
            wp_sb = wproj_pool.tile([128, KC, C], BF16)
            vr_sb = vr_pool.tile([128, NT, C], BF16)  # relu(v)*4, natural layout
            rqk_sb = rqk_pool.tile([128, 2 * NPAIR, N], BF16)  # relu(qT),relu(kT)
            outT_sb = outT_pool.tile([128, NPAIR, N], BF16)
            y1_sb = outT_pool.tile([128, KC, N], BF16)  # proj partial kc 0..4

            # ---- DMA issue order (sync HWDGE queue) ----
            # tiny consts first (the ACT warm table-load waits on b1), then
            # x in 3 chunks so transfer parallelizes across rings with the
            # qk(p0) weights following immediately.
            nc.sync.dma_start(x_sb[:, :, 0:512], xT_v[:, :, 0:512])
            nc.sync.dma_start(wfc1_sb[:], wfc1p[:, :])
            nc.sync.dma_start(wfc2_sb[:], wfc2p[:, :])
            nc.sync.dma_start(b1_sb[:], b1p[:, :])
            for m in (0, NPAIR):
                nc.sync.dma_start(wqk_sb[:, m], wqkvT_m[:, :, m, :])
            nc.sync.dma_start(x_sb[:, :, 512:N], xT_v[:, :, 512:N])
            nc.sync.dma_start(b2_sb[:], b2p[:, :])
            nc.sync.dma_start(bproj_sb[:], bprojp[:, :])
            for m in (1, NPAIR + 1):
                nc.sync.dma_start(wqk_sb[:, m], wqkvT_m[:, :, m, :])
            nc.sync.dma_start(wv_sb[:], wqkvT_v[:, :, 2 * C : 3 * C])
            for p in range(2, NPAIR):
                nc.sync.dma_start(wqk_sb[:, p], wqkvT_m[:, :, p, :])
                nc.sync.dma_start(
                    wqk_sb[:, NPAIR + p], wqkvT_m[:, :, NPAIR + p, :]
                )
            nc.sync.dma_start(wp_sb[:], wprojT_v[:, :, :])

            nc.vector.memset(ones_sb[:], 1.0)
            warmup_sb = consts.tile([128, 384], BF16)
            nc.vector.memset(warmup_sb[:], 0.0)

            # load the exp table set early (one-time ~2.7us)
            warm_sb = consts.tile([128, 2], F32)
            nc.scalar.activation(
                warm_sb[:], b1_sb[:, 0:1].to_broadcast([128, 2]),
                mybir.ActivationFunctionType.Exp,
            )

            # ---- emission helpers ----
            # All accumulators except the score tiles live in [128,512]
            # 1-bank PSUM tiles from the pv pool; score tiles are [128,1024]
            # (one head, one j-tile) cycling 3-deep for the ACT exp stream.

            def emit_qk_half(m, h):
                # one half of rows m*128..m*128+128 of qkv^T
                sl = slice(h * 512, (h + 1) * 512)
                qk_ps = pv_tile()
                for kc in range(KC):
                    nc.tensor.matmul(
                        qk_ps[:],
                        lhsT=wqk_sb[:, m, kc, :],
                        rhs=x_sb[:, kc, sl],
                        start=(kc == 0),
                        stop=(kc == KC - 1),
                    )
                nc.vector.tensor_scalar(
                    rqk_sb[:, m, sl], qk_ps[:], 0.0, None, mybir.AluOpType.max
                )

            def emit_fc(p):
                # fc1/fc2 as one 128x128 block-diagonal matmul per half
                qs_sb = spk_pool.tile([128, N], BF16, tag="spk")
                ks_sb = spk_pool.tile([128, N], BF16, tag="spk")
                for w_sb, r_m, out_sb, b_sb in (
                    (wfc1_sb, p, qs_sb, b1_sb),
                    (wfc2_sb, NPAIR + p, ks_sb, b2_sb),
                ):
                    for h in range(NH):
                        sl = slice(h * 512, (h + 1) * 512)
                        f_ps = pv_tile()
                        nc.tensor.matmul(
                            f_ps[:], lhsT=w_sb[:], rhs=rqk_sb[:, r_m, sl],
                            start=True, stop=True,
                        )
                        nc.vector.tensor_scalar(
                            out_sb[:, sl], f_ps[:], b_sb[:, 0:1], None,
                            mybir.AluOpType.add,
                        )
                return qs_sb, ks_sb

            def emit_v(nt):
                # one token tile of the v projection, natural layout
                for n0, nsz in ((0, 512), (512, 256)):
                    v_ps = pv_tile()
                    for kc in range(KC):
                        nc.tensor.matmul(
                            v_ps[:, 0:nsz],
                            lhsT=x_sb[:, kc, nt * 128 : (nt + 1) * 128],
                            rhs=wv_sb[:, kc, n0 : n0 + nsz],
                            start=(kc == 0),
                            stop=(kc == KC - 1),
                        )
                    nc.vector.tensor_scalar(
                        vr_sb[:, nt, n0 : n0 + nsz],
                        v_ps[:, 0:nsz],
                        0.0,
                        float(T_STEPS),
                        mybir.AluOpType.max,
                        mybir.AluOpType.mult,
                    )

            def emit_sc_fill(st, jt):
                # scores S^T[j,i] for one j-tile of pair p: A/B interleaved
                # per half so the 64-row tiles run concurrently (PE matmul
                # queue is strict FIFO), then exp (ACT) + max (DVE).
                qs_sb, ks_sb, pt_A, pt_B = st
                jsl = slice(jt * 128, (jt + 1) * 128)
                s_A = ps_tile()
                s_B = ps_tile()
                for h in range(NH):
                    sl = slice(h * 512, (h + 1) * 512)
                    for base, s_ps2 in ((0, s_A), (64, s_B)):
                        nc.tensor.matmul(
                            s_ps2[:, sl],
                            lhsT=ks_sb[base : base + 64, jsl],
                            rhs=qs_sb[base : base + 64, sl],
                            start=True, stop=True,
                        )
                for s_ps2, pt in ((s_A, pt_A), (s_B, pt_B)):
                    t_sb = t_pool.tile([128, N], BF16, tag="texp")
                    nc.scalar.activation(
                        t_sb[:], s_ps2[:], mybir.ActivationFunctionType.Exp
                    )
                    nc.vector.tensor_scalar(
                        pt[:, jt, :], t_sb[:], 1.0, None, mybir.AluOpType.max
                    )

            def emit_rs(p, pt_A, pt_B, eng=None):
                # row sums as ones-matmuls (64-col packing A/B) per i-half,
                # then the reciprocal reshape round trip through DRAM
                for h in range(NH):
                    sl = slice(h * 512, (h + 1) * 512)
                    rs_h = pv_tile()
                    for jt in range(NT):
                        st, sp = (jt == 0), (jt == NT - 1)
                        nc.tensor.matmul(
                            rs_h[0:64, :], lhsT=ones_sb[:],
                            rhs=pt_A[:, jt, sl], start=st, stop=sp,
                        )
                        nc.tensor.matmul(
                            rs_h[64:128, :], lhsT=ones_sb[:],
                            rhs=pt_B[:, jt, sl], start=st, stop=sp,
                        )
                    rs_rows = rs_pool.tile([128, 512], F32, tag="rsrows")
                    nc.vector.tensor_copy(
                        out=rs_rows[0:65, :], in_=rs_h[0:65, :]
                    )
                    (eng or nc.sync).dma_start(
                        rs_dram[p][:, sl], rs_rows[0:128:64, :]
                    )
                rsq = rs_pool.tile([128, 16], F32, tag="rsq")
                (eng or nc.sync).dma_start(
                    rsq[:], rs_dram[p].rearrange("h (pq t) -> h pq t", t=16)
                )
                return rsq

            def emit_recip(p, rsq, eng=None):
                # Emitted a few slabs after emit_rs so the rsq DMA has long
                # landed: an in-queue DVE wait here would stall every later
                # DVE op (memsets, max) and hiccup the whole pipeline.
                recq = rs_pool.tile([128, 16], BF16, tag="recq")
                with nc.allow_low_precision(reason="softmax denominators are O(1e3); bf16 recip adds <0.4% relative error"):
                    nc.vector.reciprocal(recq[:], rsq[:])
                (eng or nc.sync).dma_start(
                    rec_dram[p].rearrange("h (pq t) -> h pq t", t=16), recq[:]
                )
                # broadcast in bf16, split across rings to halve the latency
                recb = rs_pool.tile([128, N], BF16, tag="recb")
                for head in range(2):
                    for q in range(2):
                        (eng or nc.sync).dma_start(
                            recb[64 * head + 32 * q : 64 * head + 32 * q + 32, :],
                            rec_dram[p, head][None, :].to_broadcast([32, N]),
                        )
                return recb

            def emit_pv(p, pt_A, pt_B, recb):
                # PV product (64-col packing A/B) per i-half.  With recb the
                # normalization fuses into the PSUM->SBUF copyback; recb=None
                # copies raw so the PSUM frees without waiting the recb DMAs
                # (pair 5: normalize separately once they land).
                hA, hB = 2 * p, 2 * p + 1
                for h in range(NH):
                    sl = slice(h * 512, (h + 1) * 512)
                    out_h = pv_tile()
                    for jt in range(NT):
                        st, sp = (jt == 0), (jt == NT - 1)
                        nc.tensor.matmul(
                            out_h[0:64, :],
                            lhsT=vr_sb[:, jt, hA * HD : (hA + 1) * HD],
                            rhs=pt_A[:, jt, sl], start=st, stop=sp,
                        )
                        nc.tensor.matmul(
                            out_h[64:128, :],
                            lhsT=vr_sb[:, jt, hB * HD : (hB + 1) * HD],
                            rhs=pt_B[:, jt, sl], start=st, stop=sp,
                        )
                    if recb is not None:
                        nc.vector.tensor_tensor(
                            outT_sb[:, p, sl], out_h[:], recb[:, sl],
                            mybir.AluOpType.mult,
                        )
                    else:
                        nc.vector.tensor_copy(
                            out=outT_sb[:, p, sl], in_=out_h[:]
                        )

            def emit_proj1(et):
                # proj partial sum over kc 0..3 (pairs 0-3), bias folded in;
                # copyback on DVE (ACT is the near-critical engine).
                y_ps = ps_tile()
                for h in range(NH):
                    sl = slice(h * 512, (h + 1) * 512)
                    for kc in range(4):
                        nc.tensor.matmul(
                            y_ps[:, sl],
                            lhsT=wp_sb[:, kc, et * 128 : (et + 1) * 128],
                            rhs=outT_sb[:, kc, sl],
                            start=(kc == 0),
                            stop=(kc == 3),
                        )
                nc.vector.tensor_scalar(
                    y1_sb[:, et, :], y_ps[:], bproj_sb[:, et : et + 1], None,
                    mybir.AluOpType.add,
                )

            def emit_proj2a(et):
                # kc4 contribution (pair 4 done; independent of pair 5)
                y_ps = ps_tile()
                for h in range(NH):
                    sl = slice(h * 512, (h + 1) * 512)
                    nc.tensor.matmul(
                        y_ps[:, sl],
                        lhsT=wp_sb[:, 4, et * 128 : (et + 1) * 128],
                        rhs=outT_sb[:, 4, sl],
                        start=True,
                        stop=False,
                    )
                return y_ps

            def emit_proj2b(et, y_ps):
                # kc5 (pair 5) + the pass-1 partial, then DMA out
                for h in range(NH):
                    sl = slice(h * 512, (h + 1) * 512)
                    nc.tensor.matmul(
                        y_ps[:, sl],
                        lhsT=wp_sb[:, 5, et * 128 : (et + 1) * 128],
                        rhs=outT_sb[:, 5, sl],
                        start=False,
                        stop=True,
                    )
                y_sb = y_pool.tile([128, N], F32, tag="yt")
                nc.vector.tensor_tensor(
                    y_sb[:], y_ps[:], y1_sb[:, et, :], mybir.AluOpType.add
                )
                nc.sync.dma_start(yT_v[:, et, :], y_sb[:])

            # ---- pipelined emission ----
            # Window w = the stretch while ACT streams exp of pair w.  The
            # PE queue for window w carries the JIT score fills of pair w
            # (slot-gated by the exp drain) interleaved with slabs of work
            # whose inputs are already complete: rs/pv of pair w-1, qk/fc
            # of pair w+1, v tiles.  Costs are rough warm-clock ns used
            # only to spread slabs between fills.
            def mk_state(p):
                qs_sb, ks_sb = emit_fc(p)
                pt_A = pt_pool.tile([128, NT, N], BF16, tag="pt")
                pt_B = pt_pool.tile([128, NT, N], BF16, tag="pt")
                return (qs_sb, ks_sb, pt_A, pt_B)

            # gates: PE observes x / qk-weight / fc-weight DMA queues.
            # Later weight gates are deferred into the window slab lists so
            # they never stall the PE at startup.
            # PE warm-up: ~5us of dependency-free matmuls during the x DMA
            # wait flips the HAM clock gate to 8/8 (2.4 GHz) before the qk
            # projection starts; otherwise the whole prelude runs at 1.2.
            for _ in range(12):
                nc.tensor.matmul(
                    trash_holder[0][0:64, 0:384],
                    lhsT=warmup_sb[0:128, 0:64],
                    rhs=warmup_sb[:],
                    start=True,
                    stop=True,
                )
            gate(x_sb[:, 0, 0:512])
            for m in (0, NPAIR):
                gate(wqk_sb[:, m, 0, :])
            gate(wfc1_sb[:])
            gate(wfc2_sb[:])

            # prelude: pair-0 q,k -> fc -> first two score tiles
            emit_qk_half(0, 0)
            emit_qk_half(NPAIR, 0)
            gate(x_sb[:, 0, 512:N])
            emit_qk_half(0, 1)
            emit_qk_half(NPAIR, 1)
            state = {0: mk_state(0)}
            emit_sc_fill(state[0], 0)
            gate(wv_sb[:, 0, :])
            emit_v(0)
            emit_sc_fill(state[0], 1)
            emit_v(1)
            for m in (1, NPAIR + 1):
                gate(wqk_sb[:, m, 0, :])

            recbs = {}
            for w in range(NPAIR):
                # build this window's slab list (cost, fn)
                slabs = []
                if w == 0:
                    for nt in range(2, 6):
                        slabs.append((2400, (lambda nt=nt: emit_v(nt))))
                if w == 1:
                    for nt in range(6, NT):
                        slabs.append((2400, (lambda nt=nt: emit_v(nt))))
                if w - 1 >= 0:
                    pA, pB = state[w - 1][2], state[w - 1][3]
                    slabs.append((2300, (
                        lambda p=w - 1, a=pA, b=pB:
                        recbs.__setitem__(p, emit_rs(p, a, b)))))

                if w + 2 < NPAIR:
                    # observe next-next pair's weight DMA queues (long landed)
                    slabs.append((100, (
                        lambda m=w + 2: gate(wqk_sb[:, m, 0, :]))))
                    slabs.append((100, (
                        lambda m=NPAIR + w + 2: gate(wqk_sb[:, m, 0, :]))))
                if w == 3:
                    slabs.append((100, (lambda: gate(wp_sb[:, 0, :]))))
                if w + 1 < NPAIR:
                    for h in range(NH):
                        slabs.append((1600, (
                            lambda m=w + 1, h=h: emit_qk_half(m, h))))
                        if h == 0 and w - 1 >= 0:
                            # recip one slab after emit_rs: the qk chunk pads
                            # the DVE queue while the rsq DMA lands, and recb
                            # is still issued early enough for the PV copyback
                            slabs.append((200, (
                                lambda p=w - 1:
                                recbs.__setitem__(p, emit_recip(p, recbs[p])))))
                        slabs.append((1600, (
                            lambda m=NPAIR + w + 1, h=h: emit_qk_half(m, h))))
                    # fc + pt alloc for next pair BEFORE pv so the next
                    # window's first score fill is never blocked behind the
                    # recb-delayed PV copyback.
                    slabs.append((1000, (
                        lambda p=w + 1: state.__setitem__(p, mk_state(p)))))
                else:
                    if w - 1 >= 0:
                        slabs.append((200, (
                            lambda p=w - 1:
                            recbs.__setitem__(p, emit_recip(p, recbs[p])))))

                # pv(p) runs one window after its rs; pair 0's pv slides to
                # window 2 so windows 0/1 have room for the v projection.
                # pv0 drains first in w2: pair-2's max pass needs its pt-pool
                # slots back before exp(2) gets far.
                pv_list = [0, 1] if w == 2 else ([w - 1] if w >= 3 else [])
                for k, p in enumerate(pv_list):
                    a, b = state[p][2], state[p][3]
                    del state[p]
                    slab = (2300, (
                        lambda p=p, a=a, b=b:
                        emit_pv(p, a, b, recbs.pop(p))))
                    if w == 2 and k == 0:
                        slabs.insert(0, slab)
                    else:
                        slabs.append(slab)
                if w == NPAIR - 1:
                    # proj pass 1: kc 0..3 (pairs 0-3 all done long ago)
                    for et in range(KC):
                        slabs.append((2200, (lambda et=et: emit_proj1(et))))

                total = sum(c for c, _ in slabs)
                jts = range(2, NT) if w == 0 else range(NT)
                njt = len(jts)
                spent = 0
                done = 0.0
                for idx, jt in enumerate(jts):
                    emit_sc_fill(state[w], jt)
                    done += total / njt
                    while slabs and spent < done:
                        c, fn = slabs.pop(0)
                        fn()
                        spent += c
                for c, fn in slabs:
                    fn()

            # last pair's rowsum + PV after its exp stream drains
            # tail: pair-5 rowsum, then proj kc4 matmuls + the PV product
            # run while the pair-5 reciprocal bounces through DRAM; only the
            # PV copyback and the kc5 matmuls wait for it.
            p5 = NPAIR - 1
            pA, pB = state[p5][2], state[p5][3]
            rsq = emit_rs(p5, pA, pB, eng=nc.scalar)
            y_stage = {0: emit_proj2a(0)}
            recb = emit_recip(p5, rsq, eng=nc.scalar)
            y_stage[1] = emit_proj2a(1)
            y_stage[2] = emit_proj2a(2)
            emit_pv(p5, pA, pB, None)
            with nc.allow_low_precision(reason="bf16 in-place attention-output normalize, matches fused path precision"):
                for h in range(NH):
                    sl = slice(h * 512, (h + 1) * 512)
                    nc.vector.tensor_tensor(
                        outT_sb[:, p5, sl], outT_sb[:, p5, sl], recb[:, sl],
                        mybir.AluOpType.mult,
                    )
            for et in range(3):
                emit_proj2b(et, y_stage.pop(et))
            for et in range(3, KC):
                y_ps = emit_proj2a(et)
                emit_proj2b(et, y_ps)

    nc.compile()
    return nc


_NC_CACHE = {}


def _get_nc():
    if "nc" not in _NC_CACHE:
        _NC_CACHE["nc"] = build_nc()
    return _NC_CACHE["nc"]


def _make_in_maps(x, Wqkv, Wfc1, bfc1, Wfc2, bfc2, Wproj, bproj):
    bf = ml_dtypes.bfloat16
    s2 = 2.0 * SCALE  # fold the *SCALE and the *N_HALF accumulation into Q path
    wqkvT = np.ascontiguousarray(Wqkv.T).astype(bf)
    wfc1p = np.zeros((128, 128), np.float32)
    wfc1p[0:64, 0:64] = Wfc1.T * s2
    wfc1p[64:128, 64:128] = Wfc1.T * s2
    wfc1p = wfc1p.astype(bf)
    wfc2p = np.zeros((128, 128), np.float32)
    wfc2p[0:64, 0:64] = Wfc2.T
    wfc2p[64:128, 64:128] = Wfc2.T
    wfc2p = wfc2p.astype(bf)
    b1p = np.concatenate([bfc1 * s2, bfc1 * s2]).astype(np.float32)[:, None]
    b2p = np.concatenate([bfc2, bfc2]).astype(np.float32)[:, None]
    wprojT = np.ascontiguousarray(Wproj.T).astype(bf)
    bprojp = np.ascontiguousarray(bproj.astype(np.float32).reshape(KC, 128).T)
    shared = dict(
        wqkvT=wqkvT, wfc1p=np.ascontiguousarray(wfc1p),
        wfc2p=np.ascontiguousarray(wfc2p), b1p=b1p, b2p=b2p,
        wprojT=wprojT, bprojp=bprojp,
    )
    maps = []
    for b in range(B):
        m = dict(shared)
        m["xT"] = np.ascontiguousarray(x[b].T).astype(bf)
        maps.append(m)
    return maps


def kernel(**inputs) -> np.ndarray:
    x = np.asarray(inputs["x"], dtype=np.float32)
    nc = _get_nc()
    in_maps = _make_in_maps(
        x,
        np.asarray(inputs["Wqkv"], np.float32),
        np.asarray(inputs["Wfc1"], np.float32),
        np.asarray(inputs["bfc1"], np.float32),
        np.asarray(inputs["Wfc2"], np.float32),
        np.asarray(inputs["bfc2"], np.float32),
        np.asarray(inputs["Wproj"], np.float32),
        np.asarray(inputs["bproj"], np.float32),
    )
    res = run_bass_kernel_spmd(nc, in_maps, core_ids=list(range(B)))
    out = np.empty((B, N, C), dtype=np.float32)
    for b in range(B):
        out[b] = res.results[b]["yT"].T
    return out
